# revision 25
# baseline (speedup 1.0000x reference)
"""GroupQueryAttention (softmax over the GROUP axis) on 8 trn2 NeuronCores.

Reference computation (B=2, S=2048, D=1024, G=8, h=128):
    q = hidden @ Wq + bq ; k = hidden @ Wk + bk ; v = hidden @ Wv + bv
    scores[b,n,m,g] = sum_h q[b,n,g,h] k[b,m,g,h] / sqrt(D)
    probs = softmax(scores, axis=g)            # couples groups per (n,m)
    ctx[b,n,g,h] = sum_m probs[b,n,m,g] v[b,m,g,h]

Sharding: 2 batches x 4 query-blocks of 512 = 8 cores. The softmax over
g is local per core. Each core recomputes its batch's full K,V to avoid
cross-core collectives (~60us latency floor on this fabric).

Precision: Q,K projections run fp8e4 DoubleRowSwInterleave (weights
pre-interleaved on host so the fast-weight-load path stays on) with
x*32 / W*1024 pre-scales; V projection, scores and ctx matmuls stay
bf16 (an fp8 V or fp8 probs error enters ctx linearly through
sum_m p*dv and blows the max-abs gate).

Schedule: pass 0 (n-chunk 0) is PE-bound and carries all K/V
production, spread evenly across its 8 supertiles; its tail also
precomputes the softmax of pass 1's last NPRE supertiles into retained
E tiles (the SBUF for these is recycled from the fp8 projection
operands via a tile-pool release once projections are done). Pass 1
then runs only ST-NPRE fresh softmax chains and drains with ready ctx
matmuls instead of waiting on a chain.

Softmax runs on 2-m-tile supertiles (8 x 512 probs): exp + all PSUM
evacuations on Scalar, pair-sum tree half on GpSimd (SBUF-only engine,
otherwise idle), tree tail + normalize mul on Vector.

Host pre-lays x / x8 / wv out in the on-chip [partition, k-tile, col]
order so every input DMA is fully contiguous, split across the three
DMA-capable queues (sync, scalar, gpsimd).

Output: ctxT (1024, 512) bf16 per core; host upcasts/transposes/concats.
"""

import os

os.environ.setdefault("JAX_COMPILATION_CACHE_DIR", "/tmp/jax_comp_cache")

import numpy as np
import ml_dtypes

import concourse.bass as bass
import concourse.mybir as mybir
import concourse.tile as tile
from concourse import bacc
from concourse.bass_utils import run_bass_kernel_spmd

BF16 = mybir.dt.bfloat16
F32 = mybir.dt.float32
FP8 = mybir.dt.float8e4
DRSW = mybir.MatmulPerfMode.DoubleRowSwInterleave

B, S, D, G = 2, 2048, 1024, 8
H = D // G          # 128, group head dim
NQ = S // 4         # 512 queries per core
MT = S // 128       # 16 key m-tiles
ST = MT // 2        # 8 supertiles (2 m-tiles each)
CN = 256            # n-chunk (queries per attention pass)
NP = NQ // CN       # 2 passes
NPRE = 3            # pass-1 supertiles precomputed during pass 0
SCALE = 1.0 / np.sqrt(np.float32(D))  # 1/32
XS = 32.0           # fp8 pre-scale on x
WS = 1024.0         # fp8 pre-scale on Wq/Wk
DESC = 1.0 / (XS * WS)  # 2^-15 descale for fp8 QK psums

_CACHE = {}


def _sw_interleave(w8):
    """Host layout for DoubleRowSwInterleave stationary operands.

    w8: [128, 8, 1024] fp8 (partition, k-subtile t, out-col o). Returns
    [128, 4, 8, 256]: per (k-subtile-pair cp, out-group g of 128 cols),
    columns stored reversed with the (A=even subtile, B=odd subtile)
    values interleaved per column: pos 2*(127-c) = A[c], 2*(127-c)+1 = B[c].
    """
    A = w8[:, 0::2, :].reshape(128, 4, 8, 128)   # [p, cp, g, c]
    Bm = w8[:, 1::2, :].reshape(128, 4, 8, 128)
    inter = np.stack([A[..., ::-1], Bm[..., ::-1]], axis=-1)  # [p,cp,g,128,2]
    return np.ascontiguousarray(inter.reshape(128, 4, 8, 256))


def _build():
    nc = bacc.Bacc()

    xt_d = nc.dram_tensor("xt", [128, 8, S], BF16, kind="ExternalInput")
    xt8_d = nc.dram_tensor("xt8", [128, 8, S], FP8, kind="ExternalInput")
    wq8_d = nc.dram_tensor("wq8i", [128, 4, G, 256], FP8, kind="ExternalInput")
    wk8_d = nc.dram_tensor("wk8i", [128, 4, G, 256], FP8, kind="ExternalInput")
    wv_d = nc.dram_tensor("wv", [128, 8, D], BF16, kind="ExternalInput")
    bqs_d = nc.dram_tensor("bqs", [128, G], F32, kind="ExternalInput")
    bks_d = nc.dram_tensor("bks", [128, G], F32, kind="ExternalInput")
    bvt_d = nc.dram_tensor("bvt", [1, D], BF16, kind="ExternalInput")
    out_d = nc.dram_tensor("ctxT", [D, NQ], BF16, kind="ExternalOutput")

    with tile.TileContext(nc) as tc:
        with (
            tc.tile_pool(name="big", bufs=1) as big,
            tc.tile_pool(name="small", bufs=1) as small,
            tc.tile_pool(name="ework", bufs=3) as ework,
            tc.tile_pool(name="zwork", bufs=2) as zwork,
            tc.tile_pool(name="sc", bufs=2, space="PSUM") as scp,
            tc.tile_pool(name="cx", bufs=1, space="PSUM") as cxp,
        ):
            proj8 = tc.alloc_tile_pool(name="proj8", bufs=1)
            # ---- load inputs: contiguous layouts, 3 parallel DMA queues -----
            xt_s = big.tile([128, 8, S], BF16)       # [p, dt, m] d = dt*128+p
            xt8_s = proj8.tile([128, 8, S], FP8)
            wq8_s = proj8.tile([128, 4, G, 256], FP8)
            wk8_s = big.tile([128, 4, G, 256], FP8)
            wv_s = big.tile([128, 8, D], BF16)
            # sync queue: Q-critical then the rest of x8
            nc.sync.dma_start(xt8_s[:, :, 0:512], xt8_d[:, :, 0:512])
            nc.sync.dma_start(wq8_s[:], wq8_d[:])
            nc.sync.dma_start(xt8_s[:, :, 512:2048], xt8_d[:, :, 512:2048])
            nc.sync.dma_start(xt_s[:, :, 1024:2048], xt_d[:, :, 1024:2048])
            # scalar queue: K weights, then x bf16 for early V tiles
            nc.scalar.dma_start(wk8_s[:], wk8_d[:])
            nc.scalar.dma_start(xt_s[:, :, 0:512], xt_d[:, :, 0:512])
            nc.scalar.dma_start(xt_s[:, :, 512:1024], xt_d[:, :, 512:1024])
            # gpsimd queue: biases + V weights
            bqs_s = small.tile([128, G], F32)
            nc.gpsimd.dma_start(bqs_s[:], bqs_d[:])
            bks_s = small.tile([128, G], F32)
            nc.gpsimd.dma_start(bks_s[:], bks_d[:])
            bvt_s = small.tile([1, D], BF16)
            nc.gpsimd.dma_start(bvt_s[:], bvt_d[:])
            wv_s_dma = nc.gpsimd.dma_start(wv_s[:], wv_d[:])
            ones_s = small.tile([1, 128], BF16)
            nc.vector.memset(ones_s[:], 1.0)

            kt_s = big.tile([128, G, S], BF16)       # [h, g, m]
            v_s = big.tile([128, MT, D], BF16)       # [m, mt, g*128+h]
            qt_s = big.tile([128, G, NQ], BF16)      # [h, g, n]
            ctxt_s = big.tile([128, G, CN], BF16)    # [h, g, n] one pass

            ident = mybir.ActivationFunctionType.Identity
            expf = mybir.ActivationFunctionType.Exp

            # ---- Q^T projection (queries are XT columns 0:NQ), fp8 DRSW -----
            for g in range(G):
                qp = scp.tile([128, NQ], F32, tag="sc")
                for cp in range(4):
                    nc.tensor.matmul(
                        qp[:],
                        wq8_s[:, cp, g, :],
                        xt8_s[:, 2 * cp : 2 * cp + 2, 0:NQ],
                        start=(cp == 0),
                        stop=(cp == 3),
                        perf_mode=DRSW,
                    )
                nc.scalar.activation(
                    qt_s[:, g, :], qp[:], ident,
                    bias=bqs_s[:, g : g + 1], scale=float(SCALE * DESC),
                )

            # ---- K/V production pieces --------------------------------------
            def k_part(mc, gh):
                # K^T columns mc*512 .. +512 for groups gh*4..+4: fp8
                # DoubleRow with software-interleaved weights (keeps FWL)
                for g in range(gh * 4, gh * 4 + 4):
                    kp = scp.tile([128, 512], F32, tag="sc")
                    for cp in range(4):
                        nc.tensor.matmul(
                            kp[:],
                            wk8_s[:, cp, g, :],
                            xt8_s[:, 2 * cp : 2 * cp + 2,
                                  mc * 512 : (mc + 1) * 512],
                            start=(cp == 0),
                            stop=(cp == 3),
                            perf_mode=DRSW,
                        )
                    nc.scalar.activation(
                        kt_s[:, g, mc * 512 : (mc + 1) * 512], kp[:], ident,
                        bias=bks_s[:, g : g + 1], scale=float(DESC),
                    )

            def v_mt(mt):
                # V rows for one m-tile of 128 x full D
                for hc in range(2):
                    vp = scp.tile([128, 512], F32, tag="sc")
                    for dt in range(8):
                        nc.tensor.matmul(
                            vp[:],
                            xt_s[:, dt, mt * 128 : (mt + 1) * 128],
                            wv_s[:, dt, hc * 512 : (hc + 1) * 512],
                            start=(dt == 0),
                            stop=False,
                        )
                    nc.tensor.matmul(
                        vp[:],
                        ones_s[:],
                        bvt_s[:, hc * 512 : (hc + 1) * 512],
                        start=False,
                        stop=True,
                    )
                    nc.scalar.activation(
                        v_s[:, mt, hc * 512 : (hc + 1) * 512], vp[:], ident
                    )

            def scores_softmax(np_, st, fill=None, pool=None, tag="e"):
                """Supertile: scores + exp + group-softmax for m-tiles
                2*st and 2*st+1 against n-chunk np_. E layout
                [128, g, sub*256+n]. fill(i) emits deferred PE work after
                each of the 4 exp stages. Returns the normalized E tile."""
                n0 = np_ * CN
                e_s = (pool or ework).tile([128, G, 2 * CN], BF16, tag=tag)
                t1 = zwork.tile([128, 4, 2 * CN], BF16, tag="t1", bufs=1)
                # stage order (sub, half): (0,0),(1,0),(0,1),(1,1) so the
                # half-0 pair-sum (gpsimd) can start at the halfway point
                for i in range(4):
                    half, sub = divmod(i, 2)
                    mt = 2 * st + sub
                    sp = scp.tile([128, 4, CN], F32, tag="sc")
                    for gl in range(4):
                        g = half * 4 + gl
                        nc.tensor.matmul(
                            sp[:, gl, :],
                            kt_s[:, g, mt * 128 : (mt + 1) * 128],
                            qt_s[:, g, n0 : n0 + CN],
                            start=True,
                            stop=True,
                        )
                    nc.scalar.activation(
                        e_s[:, half * 4 : half * 4 + 4, sub * CN : (sub + 1) * CN],
                        sp[:], expf,
                    )
                    # pair-sums: half 0 on gpsimd (slack before t2 needs it),
                    # half 1 on vector (fast, feeds t2 immediately)
                    if i == 1:
                        nc.gpsimd.tensor_add(
                            t1[:, 0:2, :], e_s[:, 0:2, :], e_s[:, 2:4, :]
                        )
                    elif i == 3:
                        nc.vector.tensor_add(
                            t1[:, 2:4, :], e_s[:, 4:6, :], e_s[:, 6:8, :]
                        )
                    if fill is not None:
                        fill(i)
                t2 = zwork.tile([128, 2, 2 * CN], BF16, tag="t2", bufs=1)
                nc.vector.tensor_add(t2[:], t1[:, 0:2, :], t1[:, 2:4, :])
                z32 = zwork.tile([128, 2 * CN], F32, tag="z32", bufs=1)
                nc.vector.tensor_add(z32[:], t2[:, 0, :], t2[:, 1, :])
                nc.vector.reciprocal_approx_fast(out=z32[:], in_=z32[:])
                wb = zwork.tile([128, 2 * CN], BF16, tag="wb", bufs=1)
                nc.vector.tensor_copy(wb[:], z32[:])
                # normalize per sub-tile so ctx matmuls on sub 0 can start
                # while sub 1 is still being scaled
                for sub in range(2):
                    wb_b = bass.AP(
                        tensor=wb.tensor, offset=wb.offset + sub * CN,
                        ap=[wb.ap[0], [0, G], [1, CN]],
                    )
                    nc.vector.tensor_mul(
                        e_s[:, :, sub * CN : (sub + 1) * CN],
                        e_s[:, :, sub * CN : (sub + 1) * CN],
                        wb_b,
                    )
                return e_s

            def ctx_mms(st, e_s, ctx_acc, sub, gs=range(G), stop=False):
                # ctx^T accumulation: out[h, n] += V_g^T @ P_g^T
                # Two groups share each 2KB PSUM bank. start=True resets the
                # whole bank's has_written bits, so only the first group of
                # each bank pair may issue it; the second group's first write
                # lands on cleared bits and overwrites, later writes accumulate.
                mt = 2 * st + sub
                for g in gs:
                    nc.tensor.matmul(
                        ctx_acc[:, g, :],
                        v_s[:, mt, g * 128 : (g + 1) * 128],
                        e_s[:, g, sub * CN : (sub + 1) * CN],
                        start=(mt == 0 and g % 2 == 0),
                        stop=stop,
                        skip_group_check=True,
                    )

            def drain_and_evac(np_, ctx_acc, pend, pre):
                # drain order: all ready supertiles first, the freshest
                # (whose softmax chain may still be running) last, so the
                # PE always has ready ctx work while the last chain ends.
                order = pend[:-1] + sorted(pre.items()) + pend[-1:]
                n0 = np_ * CN
                out_r = out_d.rearrange("(t p) n -> p t n", p=128)
                for gh in range(2):
                    gs = range(gh * 4, gh * 4 + 4)
                    for st_, e_ in order:
                        for sub in range(2):
                            nc_stop = (st_, sub) == (order[-1][0], 1)
                            ctx_mms(st_, e_, ctx_acc, sub, gs=gs, stop=nc_stop)
                    nc.scalar.activation(
                        ctxt_s[:, gh * 4 : gh * 4 + 4, :],
                        ctx_acc[:, gh * 4 : gh * 4 + 4, :], ident,
                    )
                    nc.sync.dma_start(
                        out_r[:, gh * 4 : gh * 4 + 4, n0 : n0 + CN],
                        ctxt_s[:, gh * 4 : gh * 4 + 4, :],
                    )

            def emit_pass(np_, ctx_acc, with_kv, n_fresh, pre, pre_out):
                # software pipeline over supertiles: ctx matmuls for
                # supertile st are emitted across the 4 exp stages of
                # supertile st+depth, hiding the exp->tree->mul chain.
                # In pass 0 the K/V production is spread evenly: each
                # supertile produces its own 2 V m-tiles plus half the
                # groups of the NEXT 512-col K chunk (consumed 2 supers
                # later). Pass 0 supertiles >= ST-NPRE also precompute
                # pass 1's softmax into retained E tiles.
                depth = 1 if with_kv else 2
                pend = []
                for st in range(n_fresh):
                    if with_kv:
                        v_mt(2 * st)
                        if st < 6:
                            k_part(st // 2 + 1, st % 2)
                        v_mt(2 * st + 1)
                        if st == 5:
                            # fp8 projection operands are dead once the
                            # last K part is emitted; recycle their SBUF
                            # for the retained pass-1 E tiles.
                            proj8.release()
                            pre_out["pool"] = tc.alloc_tile_pool(
                                name="epre", bufs=NPRE
                            )

                    def fill(i, _p=(pend[0] if len(pend) == depth else None)):
                        if _p is not None:
                            half, sub = divmod(i, 2)
                            ctx_mms(_p[0], _p[1], ctx_acc, sub,
                                    gs=range(half * 4, half * 4 + 4))

                    e_s = scores_softmax(np_, st, fill)
                    if len(pend) == depth:
                        pend.pop(0)
                    pend.append((st, e_s))
                    if pre_out is not None and st >= ST - NPRE:
                        pre_out[st] = scores_softmax(
                            1, st, pool=pre_out["pool"], tag="epre"
                        )
                return pend

            # prologue: K chunk 0 so pass-0 supertile 0 can score
            k_part(0, 0)
            k_part(0, 1)
            epre = {}
            ctx_acc = cxp.tile([128, G, CN], F32, tag="cx")
            pend = emit_pass(0, ctx_acc, True, ST, {}, epre)
            epool = epre.pop("pool")
            drain_and_evac(0, ctx_acc, pend, {})
            ctx_acc = cxp.tile([128, G, CN], F32, tag="cx")
            pend = emit_pass(1, ctx_acc, False, ST - NPRE, epre, None)
            drain_and_evac(1, ctx_acc, pend, epre)
            epool.release()

    nc.compile()
    return nc


def _prep_inputs(hidden_states, Wq, bq, Wk, bk, Wv, bv):
    bf = ml_dtypes.bfloat16
    f8 = ml_dtypes.float8_e4m3
    # wv rearranged to the on-chip [p, t, o] layout (d = t*128 + p)
    wv_b = np.ascontiguousarray(
        np.asarray(Wv, np.float32).reshape(8, 128, D).transpose(1, 0, 2)
    ).astype(bf)

    # Wq/Wk scaled fp8, rearranged [d, o] -> [p, t, o], then
    # software-interleaved for DoubleRowSwInterleave
    def prep_w8(W):
        w8 = (np.asarray(W, np.float32) * WS).astype(f8)
        return _sw_interleave(w8.reshape(8, 128, D).transpose(1, 0, 2))

    wq8i = prep_w8(Wq)
    wk8i = prep_w8(Wk)
    bqs = np.ascontiguousarray(
        (np.asarray(bq, np.float32) * SCALE).reshape(G, 128).T
    )
    bks = np.ascontiguousarray(np.asarray(bk, np.float32).reshape(G, 128).T)
    bvt = np.asarray(bv, np.float32).astype(bf).reshape(1, D)

    in_maps = []
    for core in range(8):
        b, j = divmod(core, 4)
        xt = np.asarray(hidden_states[b], np.float32).T  # (D, S)
        xt = np.roll(xt, -j * NQ, axis=1)                # queries first
        xtp = np.ascontiguousarray(
            xt.reshape(8, 128, S).transpose(1, 0, 2)     # [p, t, m]
        )
        in_maps.append(
            {
                "xt": xtp.astype(bf),
                "xt8": (xtp * XS).astype(f8),
                "wq8i": wq8i, "wk8i": wk8i, "wv": wv_b,
                "bqs": bqs, "bks": bks, "bvt": bvt,
            }
        )
    return in_maps


def kernel(hidden_states, Wq, bq, Wk, bk, Wv, bv, _trace=False, _tmpdir=None):
    if "nc" not in _CACHE:
        _CACHE["nc"] = _build()
    nc = _CACHE["nc"]
    in_maps = _prep_inputs(hidden_states, Wq, bq, Wk, bk, Wv, bv)
    res = run_bass_kernel_spmd(
        nc, in_maps, list(range(8)), trace=_trace,
        **({"tmpdir": _tmpdir} if _tmpdir else {}),
    )
    _CACHE["last_result"] = res
    out = np.empty((B, S, D), np.float32)
    for core in range(8):
        b, j = divmod(core, 4)
        out[b, j * NQ : (j + 1) * NQ, :] = (
            res.results[core]["ctxT"].astype(np.float32).T
        )
    return out


# revision 31
# speedup vs baseline: 1.0242x; 1.0242x over previous
"""GroupQueryAttention (softmax over the GROUP axis) on 8 trn2 NeuronCores.

Reference computation (B=2, S=2048, D=1024, G=8, h=128):
    q = hidden @ Wq + bq ; k = hidden @ Wk + bk ; v = hidden @ Wv + bv
    scores[b,n,m,g] = sum_h q[b,n,g,h] k[b,m,g,h] / sqrt(D)
    probs = softmax(scores, axis=g)            # couples groups per (n,m)
    ctx[b,n,g,h] = sum_m probs[b,n,m,g] v[b,m,g,h]

Sharding: 2 batches x 4 query-blocks of 512 = 8 cores. The softmax over
g is local per core. Each core recomputes its batch's full K,V to avoid
cross-core collectives (~60us latency floor on this fabric).

Precision: Q,K projections run fp8e4 DoubleRowSwInterleave (weights
pre-interleaved on host so the fast-weight-load path stays on) with
x*32 / W*1024 pre-scales; V projection, scores and ctx matmuls stay
bf16 (an fp8 V or fp8 probs error enters ctx linearly through
sum_m p*dv and blows the max-abs gate).

Schedule: pass 0 (n-chunk 0) is PE-bound and carries all K/V
production, spread evenly across its 8 supertiles; its tail also
precomputes the softmax of pass 1's last NPRE supertiles into retained
E tiles (the SBUF for these is recycled from the fp8 projection
operands via a tile-pool release once projections are done). Pass 1
then runs only ST-NPRE fresh softmax chains and drains with ready ctx
matmuls instead of waiting on a chain.

Softmax runs on 2-m-tile supertiles (8 x 512 probs): exp + all PSUM
evacuations on Scalar, pair-sum tree half on GpSimd (SBUF-only engine,
otherwise idle), tree tail + normalize mul on Vector.

Host pre-lays x / x8 / wv out in the on-chip [partition, k-tile, col]
order so every input DMA is fully contiguous, split across the three
DMA-capable queues (sync, scalar, gpsimd).

Output: ctxT (1024, 512) bf16 per core; host upcasts/transposes/concats.
"""

import os

os.environ.setdefault("JAX_COMPILATION_CACHE_DIR", "/tmp/jax_comp_cache")

import numpy as np
import ml_dtypes

import concourse.bass as bass
import concourse.mybir as mybir
import concourse.tile as tile
from concourse import bacc
from concourse.bass_utils import run_bass_kernel_spmd

BF16 = mybir.dt.bfloat16
F32 = mybir.dt.float32
FP8 = mybir.dt.float8e4
DRSW = mybir.MatmulPerfMode.DoubleRowSwInterleave

B, S, D, G = 2, 2048, 1024, 8
H = D // G          # 128, group head dim
NQ = S // 4         # 512 queries per core
MT = S // 128       # 16 key m-tiles
ST = MT // 2        # 8 supertiles (2 m-tiles each)
CN = 256            # n-chunk (queries per attention pass)
NP = NQ // CN       # 2 passes
NPRE = 3            # pass-1 supertiles precomputed during pass 0
SCALE = 1.0 / np.sqrt(np.float32(D))  # 1/32
XS = 32.0           # fp8 pre-scale on x
WS = 1024.0         # fp8 pre-scale on Wq/Wk
DESC = 1.0 / (XS * WS)  # 2^-15 descale for fp8 QK psums

_CACHE = {}


def _sw_interleave(w8):
    """Host layout for DoubleRowSwInterleave stationary operands.

    w8: [128, 8, 1024] fp8 (partition, k-subtile t, out-col o). Returns
    [128, 4, 8, 256]: per (k-subtile-pair cp, out-group g of 128 cols),
    columns stored reversed with the (A=even subtile, B=odd subtile)
    values interleaved per column: pos 2*(127-c) = A[c], 2*(127-c)+1 = B[c].
    """
    A = w8[:, 0::2, :].reshape(128, 4, 8, 128)   # [p, cp, g, c]
    Bm = w8[:, 1::2, :].reshape(128, 4, 8, 128)
    inter = np.stack([A[..., ::-1], Bm[..., ::-1]], axis=-1)  # [p,cp,g,128,2]
    return np.ascontiguousarray(inter.reshape(128, 4, 8, 256))


def _build():
    nc = bacc.Bacc()

    xt_d = nc.dram_tensor("xt", [4, 128, 8, 512], BF16, kind="ExternalInput")
    xt8_d = nc.dram_tensor("xt8", [4, 128, 8, 512], FP8, kind="ExternalInput")
    wq8_d = nc.dram_tensor("wq8i", [128, 4, G, 256], FP8, kind="ExternalInput")
    wk8_d = nc.dram_tensor("wk8i", [128, 4, G, 256], FP8, kind="ExternalInput")
    wv_d = nc.dram_tensor("wv", [128, 8, D], BF16, kind="ExternalInput")
    bqs_d = nc.dram_tensor("bqs", [128, G], F32, kind="ExternalInput")
    bks_d = nc.dram_tensor("bks", [128, G], F32, kind="ExternalInput")
    bvt_d = nc.dram_tensor("bvt", [1, D], BF16, kind="ExternalInput")
    out_d = nc.dram_tensor("ctxT", [NP, 2, 128, 4, CN], BF16,
                           kind="ExternalOutput")

    with tile.TileContext(nc) as tc:
        with (
            tc.tile_pool(name="big", bufs=1) as big,
            tc.tile_pool(name="small", bufs=1) as small,
            tc.tile_pool(name="ework", bufs=3) as ework,
            tc.tile_pool(name="zwork", bufs=2) as zwork,
            tc.tile_pool(name="sc", bufs=2, space="PSUM") as scp,
            tc.tile_pool(name="cx", bufs=1, space="PSUM") as cxp,
        ):
            proj8 = tc.alloc_tile_pool(name="proj8", bufs=1)
            # ---- load inputs: chunk-major contiguous layouts, 3 queues ------
            xt_s = big.tile([128, 4, 8, 512], BF16)  # [p, mc, dt, mcol]
            xt8_s = proj8.tile([128, 4, 8, 512], FP8)
            wq8_s = proj8.tile([128, 4, G, 256], FP8)
            wk8_s = big.tile([128, 4, G, 256], FP8)
            wv_s = big.tile([128, 8, D], BF16)
            # sync queue: Q-critical then the rest of x8
            nc.sync.dma_start(xt8_s[:, 0], xt8_d[0])
            nc.sync.dma_start(wq8_s[:], wq8_d[:])
            for c in range(1, 4):
                nc.sync.dma_start(xt8_s[:, c], xt8_d[c])
            nc.sync.dma_start(xt_s[:, 2], xt_d[2])
            nc.sync.dma_start(xt_s[:, 3], xt_d[3])
            # scalar queue: K weights, then x bf16 for early V tiles
            nc.scalar.dma_start(wk8_s[:], wk8_d[:])
            nc.scalar.dma_start(xt_s[:, 0], xt_d[0])
            nc.scalar.dma_start(xt_s[:, 1], xt_d[1])
            # gpsimd queue: biases + V weights
            bqs_s = small.tile([128, G], F32)
            nc.gpsimd.dma_start(bqs_s[:], bqs_d[:])
            bks_s = small.tile([128, G], F32)
            nc.gpsimd.dma_start(bks_s[:], bks_d[:])
            bvt_s = small.tile([1, D], BF16)
            nc.gpsimd.dma_start(bvt_s[:], bvt_d[:])
            wv_s_dma = nc.gpsimd.dma_start(wv_s[:], wv_d[:])
            ones_s = small.tile([1, 128], BF16)
            nc.vector.memset(ones_s[:], 1.0)

            kt_s = big.tile([128, G, S], BF16)       # [h, g, m]
            v_s = big.tile([128, MT, D], BF16)       # [m, mt, g*128+h]
            qt_s = big.tile([128, G, NQ], BF16)      # [h, g, n]
            ctxt_s = big.tile([128, G, CN], BF16)    # [h, g, n] one pass

            ident = mybir.ActivationFunctionType.Identity
            expf = mybir.ActivationFunctionType.Exp

            # ---- Q^T projection (queries are XT columns 0:NQ), fp8 DRSW -----
            for g in range(G):
                qp = scp.tile([128, NQ], F32, tag="sc")
                for cp in range(4):
                    nc.tensor.matmul(
                        qp[:],
                        wq8_s[:, cp, g, :],
                        xt8_s[:, 0, 2 * cp : 2 * cp + 2, :],
                        start=(cp == 0),
                        stop=(cp == 3),
                        perf_mode=DRSW,
                    )
                nc.scalar.activation(
                    qt_s[:, g, :], qp[:], ident,
                    bias=bqs_s[:, g : g + 1], scale=float(SCALE * DESC),
                )

            # ---- K/V production pieces --------------------------------------
            def k_part(mc, gh):
                # K^T columns mc*512 .. +512 for groups gh*4..+4: fp8
                # DoubleRow with software-interleaved weights (keeps FWL)
                for g in range(gh * 4, gh * 4 + 4):
                    kp = scp.tile([128, 512], F32, tag="sc")
                    for cp in range(4):
                        nc.tensor.matmul(
                            kp[:],
                            wk8_s[:, cp, g, :],
                            xt8_s[:, mc, 2 * cp : 2 * cp + 2, :],
                            start=(cp == 0),
                            stop=(cp == 3),
                            perf_mode=DRSW,
                        )
                    nc.scalar.activation(
                        kt_s[:, g, mc * 512 : (mc + 1) * 512], kp[:], ident,
                        bias=bks_s[:, g : g + 1], scale=float(DESC),
                    )

            def v_mt(mt):
                # V rows for one m-tile of 128 x full D
                for hc in range(2):
                    vp = scp.tile([128, 512], F32, tag="sc")
                    for dt in range(8):
                        nc.tensor.matmul(
                            vp[:],
                            xt_s[:, mt // 4, dt,
                                 (mt % 4) * 128 : (mt % 4) * 128 + 128],
                            wv_s[:, dt, hc * 512 : (hc + 1) * 512],
                            start=(dt == 0),
                            stop=False,
                        )
                    nc.tensor.matmul(
                        vp[:],
                        ones_s[:],
                        bvt_s[:, hc * 512 : (hc + 1) * 512],
                        start=False,
                        stop=True,
                    )
                    nc.scalar.activation(
                        v_s[:, mt, hc * 512 : (hc + 1) * 512], vp[:], ident
                    )

            def scores_softmax(np_, st, fill=None, pool=None, tag="e"):
                """Supertile: scores + exp + group-softmax for m-tiles
                2*st and 2*st+1 against n-chunk np_. E layout
                [128, g, sub*256+n]. fill(i) emits deferred PE work after
                each of the 4 exp stages. Returns the normalized E tile."""
                n0 = np_ * CN
                e_s = (pool or ework).tile([128, G, 2 * CN], BF16, tag=tag)
                t1 = zwork.tile([128, 4, 2 * CN], BF16, tag="t1", bufs=1)
                # stage order (sub, half): (0,0),(1,0),(0,1),(1,1) so the
                # half-0 pair-sum (gpsimd) can start at the halfway point
                for i in range(4):
                    half, sub = divmod(i, 2)
                    mt = 2 * st + sub
                    sp = scp.tile([128, 4, CN], F32, tag="sc")
                    for gl in range(4):
                        g = half * 4 + gl
                        nc.tensor.matmul(
                            sp[:, gl, :],
                            kt_s[:, g, mt * 128 : (mt + 1) * 128],
                            qt_s[:, g, n0 : n0 + CN],
                            start=True,
                            stop=True,
                        )
                    nc.scalar.activation(
                        e_s[:, half * 4 : half * 4 + 4, sub * CN : (sub + 1) * CN],
                        sp[:], expf,
                    )
                    # pair-sums: half 0 on gpsimd (slack before t2 needs it),
                    # half 1 on vector (fast, feeds t2 immediately)
                    if i == 1:
                        nc.gpsimd.tensor_add(
                            t1[:, 0:2, :], e_s[:, 0:2, :], e_s[:, 2:4, :]
                        )
                    elif i == 3:
                        nc.vector.tensor_add(
                            t1[:, 2:4, :], e_s[:, 4:6, :], e_s[:, 6:8, :]
                        )
                    if fill is not None:
                        fill(i)
                t2 = zwork.tile([128, 2, 2 * CN], BF16, tag="t2", bufs=1)
                nc.vector.tensor_add(t2[:], t1[:, 0:2, :], t1[:, 2:4, :])
                z32 = zwork.tile([128, 2 * CN], F32, tag="z32", bufs=1)
                nc.vector.tensor_add(z32[:], t2[:, 0, :], t2[:, 1, :])
                nc.vector.reciprocal_approx_fast(out=z32[:], in_=z32[:])
                wb = zwork.tile([128, 2 * CN], BF16, tag="wb", bufs=1)
                nc.vector.tensor_copy(wb[:], z32[:])
                # normalize per sub-tile so ctx matmuls on sub 0 can start
                # while sub 1 is still being scaled
                for sub in range(2):
                    wb_b = bass.AP(
                        tensor=wb.tensor, offset=wb.offset + sub * CN,
                        ap=[wb.ap[0], [0, G], [1, CN]],
                    )
                    nc.vector.tensor_mul(
                        e_s[:, :, sub * CN : (sub + 1) * CN],
                        e_s[:, :, sub * CN : (sub + 1) * CN],
                        wb_b,
                    )
                return e_s

            def ctx_mms(st, e_s, ctx_acc, sub, gs=range(G), stop=False):
                # ctx^T accumulation: out[h, n] += V_g^T @ P_g^T
                # Two groups share each 2KB PSUM bank. start=True resets the
                # whole bank's has_written bits, so only the first group of
                # each bank pair may issue it; the second group's first write
                # lands on cleared bits and overwrites, later writes accumulate.
                mt = 2 * st + sub
                for g in gs:
                    nc.tensor.matmul(
                        ctx_acc[:, g, :],
                        v_s[:, mt, g * 128 : (g + 1) * 128],
                        e_s[:, g, sub * CN : (sub + 1) * CN],
                        start=(mt == 0 and g % 2 == 0),
                        stop=stop,
                        skip_group_check=True,
                    )

            def drain_and_evac(np_, ctx_acc, pend, pre):
                # drain order: all ready supertiles first, the freshest
                # (whose softmax chain may still be running) last, so the
                # PE always has ready ctx work while the last chain ends.
                order = pend[:-1] + sorted(pre.items()) + pend[-1:]
                for gh in range(2):
                    gs = range(gh * 4, gh * 4 + 4)
                    for st_, e_ in order:
                        for sub in range(2):
                            nc_stop = (st_, sub) == (order[-1][0], 1)
                            ctx_mms(st_, e_, ctx_acc, sub, gs=gs, stop=nc_stop)
                    nc.scalar.activation(
                        ctxt_s[:, gh * 4 : gh * 4 + 4, :],
                        ctx_acc[:, gh * 4 : gh * 4 + 4, :], ident,
                    )
                    nc.sync.dma_start(
                        out_d[np_, gh], ctxt_s[:, gh * 4 : gh * 4 + 4, :]
                    )

            def emit_pass(np_, ctx_acc, with_kv, n_fresh, pre, pre_out):
                # software pipeline over supertiles: ctx matmuls for
                # supertile st are emitted across the 4 exp stages of
                # supertile st+depth, hiding the exp->tree->mul chain.
                # In pass 0 the K/V production is spread evenly: each
                # supertile produces its own 2 V m-tiles plus half the
                # groups of the NEXT 512-col K chunk (consumed 2 supers
                # later). Pass 0 supertiles >= ST-NPRE also precompute
                # pass 1's softmax into retained E tiles.
                depth = 1 if with_kv else 2
                pend = []
                for st in range(n_fresh):
                    if with_kv:
                        v_mt(2 * st)
                        if st < 6:
                            k_part(st // 2 + 1, st % 2)
                        v_mt(2 * st + 1)
                        if st == 5:
                            # fp8 projection operands are dead once the
                            # last K part is emitted; recycle their SBUF
                            # for the retained pass-1 E tiles.
                            proj8.release()
                            pre_out["pool"] = tc.alloc_tile_pool(
                                name="epre", bufs=NPRE
                            )

                    def fill(i, _p=(pend[0] if len(pend) == depth else None)):
                        if _p is not None:
                            half, sub = divmod(i, 2)
                            ctx_mms(_p[0], _p[1], ctx_acc, sub,
                                    gs=range(half * 4, half * 4 + 4))

                    e_s = scores_softmax(np_, st, fill)
                    if len(pend) == depth:
                        pend.pop(0)
                    pend.append((st, e_s))
                    if pre_out is not None and st >= ST - NPRE:
                        pre_out[st] = scores_softmax(
                            1, st, pool=pre_out["pool"], tag="epre"
                        )
                return pend

            # prologue: K chunk 0 so pass-0 supertile 0 can score
            k_part(0, 0)
            k_part(0, 1)
            epre = {}
            ctx_acc = cxp.tile([128, G, CN], F32, tag="cx")
            pend = emit_pass(0, ctx_acc, True, ST, {}, epre)
            epool = epre.pop("pool")
            drain_and_evac(0, ctx_acc, pend, {})
            ctx_acc = cxp.tile([128, G, CN], F32, tag="cx")
            pend = emit_pass(1, ctx_acc, False, ST - NPRE, epre, None)
            drain_and_evac(1, ctx_acc, pend, epre)
            epool.release()

    nc.compile()
    return nc


def _prep_inputs(hidden_states, Wq, bq, Wk, bk, Wv, bv):
    bf = ml_dtypes.bfloat16
    f8 = ml_dtypes.float8_e4m3
    # wv rearranged to the on-chip [p, t, o] layout (d = t*128 + p)
    wv_b = np.ascontiguousarray(
        np.asarray(Wv, np.float32).reshape(8, 128, D).transpose(1, 0, 2)
    ).astype(bf)

    # Wq/Wk scaled fp8, rearranged [d, o] -> [p, t, o], then
    # software-interleaved for DoubleRowSwInterleave
    def prep_w8(W):
        w8 = (np.asarray(W, np.float32) * WS).astype(f8)
        return _sw_interleave(w8.reshape(8, 128, D).transpose(1, 0, 2))

    wq8i = prep_w8(Wq)
    wk8i = prep_w8(Wk)
    bqs = np.ascontiguousarray(
        (np.asarray(bq, np.float32) * SCALE).reshape(G, 128).T
    )
    bks = np.ascontiguousarray(np.asarray(bk, np.float32).reshape(G, 128).T)
    bvt = np.asarray(bv, np.float32).astype(bf).reshape(1, D)

    in_maps = []
    for core in range(8):
        b, j = divmod(core, 4)
        xt = np.asarray(hidden_states[b], np.float32).T  # (D, S)
        xt = np.roll(xt, -j * NQ, axis=1)                # queries first
        # chunk-major on-chip layout [mc, p, t, mcol] (d = t*128 + p)
        xtp = np.ascontiguousarray(
            xt.reshape(8, 128, 4, 512).transpose(2, 1, 0, 3)
        )
        in_maps.append(
            {
                "xt": xtp.astype(bf),
                "xt8": (xtp * XS).astype(f8),
                "wq8i": wq8i, "wk8i": wk8i, "wv": wv_b,
                "bqs": bqs, "bks": bks, "bvt": bvt,
            }
        )
    return in_maps


def kernel(hidden_states, Wq, bq, Wk, bk, Wv, bv, _trace=False, _tmpdir=None):
    if "nc" not in _CACHE:
        _CACHE["nc"] = _build()
    nc = _CACHE["nc"]
    in_maps = _prep_inputs(hidden_states, Wq, bq, Wk, bk, Wv, bv)
    res = run_bass_kernel_spmd(
        nc, in_maps, list(range(8)), trace=_trace,
        **({"tmpdir": _tmpdir} if _tmpdir else {}),
    )
    _CACHE["last_result"] = res
    out = np.empty((B, S, D), np.float32)
    for core in range(8):
        b, j = divmod(core, 4)
        # ctxT [np, gh, p, gl, n]: d = (gh*4+gl)*128 + p, row = np*CN + n
        ct = res.results[core]["ctxT"].astype(np.float32)
        blk = ct.transpose(0, 4, 1, 3, 2).reshape(NQ, D)
        out[b, j * NQ : (j + 1) * NQ, :] = blk
    return out


# revision 34
# speedup vs baseline: 1.0274x; 1.0031x over previous
"""GroupQueryAttention (softmax over the GROUP axis) on 8 trn2 NeuronCores.

Reference computation (B=2, S=2048, D=1024, G=8, h=128):
    q = hidden @ Wq + bq ; k = hidden @ Wk + bk ; v = hidden @ Wv + bv
    scores[b,n,m,g] = sum_h q[b,n,g,h] k[b,m,g,h] / sqrt(D)
    probs = softmax(scores, axis=g)            # couples groups per (n,m)
    ctx[b,n,g,h] = sum_m probs[b,n,m,g] v[b,m,g,h]

Sharding: 2 batches x 4 query-blocks of 512 = 8 cores. The softmax over
g is local per core. Each core recomputes its batch's full K,V to avoid
cross-core collectives (~60us latency floor on this fabric).

Precision: Q,K projections run fp8e4 DoubleRowSwInterleave (weights
pre-interleaved on host so the fast-weight-load path stays on) with
x*32 / W*1024 pre-scales; V projection, scores and ctx matmuls stay
bf16 (an fp8 V or fp8 probs error enters ctx linearly through
sum_m p*dv and blows the max-abs gate).

Schedule: pass 0 (n-chunk 0) is PE-bound and carries all K/V
production, spread evenly across its 8 supertiles; its tail also
precomputes the softmax of pass 1's last NPRE supertiles into retained
E tiles (the SBUF for these is recycled from the fp8 projection
operands via a tile-pool release once projections are done). Pass 1
then runs only ST-NPRE fresh softmax chains and drains with ready ctx
matmuls instead of waiting on a chain.

Softmax runs on 2-m-tile supertiles (8 x 512 probs): exp + all PSUM
evacuations on Scalar, pair-sum tree half on GpSimd (SBUF-only engine,
otherwise idle), tree tail + normalize mul on Vector.

Host pre-lays x / x8 / wv out in the on-chip [partition, k-tile, col]
order so every input DMA is fully contiguous, split across the three
DMA-capable queues (sync, scalar, gpsimd).

Output: ctxT (1024, 512) bf16 per core; host upcasts/transposes/concats.
"""

import os

os.environ.setdefault("JAX_COMPILATION_CACHE_DIR", "/tmp/jax_comp_cache")

import numpy as np
import ml_dtypes

import concourse.bass as bass
import concourse.mybir as mybir
import concourse.tile as tile
from concourse import bacc
from concourse.bass_utils import run_bass_kernel_spmd

BF16 = mybir.dt.bfloat16
F32 = mybir.dt.float32
FP8 = mybir.dt.float8e4
DRSW = mybir.MatmulPerfMode.DoubleRowSwInterleave

B, S, D, G = 2, 2048, 1024, 8
H = D // G          # 128, group head dim
NQ = S // 4         # 512 queries per core
MT = S // 128       # 16 key m-tiles
ST = MT // 2        # 8 supertiles (2 m-tiles each)
CN = 256            # n-chunk (queries per attention pass)
NP = NQ // CN       # 2 passes
NPRE = 4            # pass-1 supertiles precomputed during pass 0
SCALE = 1.0 / np.sqrt(np.float32(D))  # 1/32
XS = 32.0           # fp8 pre-scale on x
WS = 1024.0         # fp8 pre-scale on Wq/Wk
DESC = 1.0 / (XS * WS)  # 2^-15 descale for fp8 QK psums

_CACHE = {}


def _sw_interleave(w8):
    """Host layout for DoubleRowSwInterleave stationary operands.

    w8: [128, 8, 1024] fp8 (partition, k-subtile t, out-col o). Returns
    [128, 4, 8, 256]: per (k-subtile-pair cp, out-group g of 128 cols),
    columns stored reversed with the (A=even subtile, B=odd subtile)
    values interleaved per column: pos 2*(127-c) = A[c], 2*(127-c)+1 = B[c].
    """
    A = w8[:, 0::2, :].reshape(128, 4, 8, 128)   # [p, cp, g, c]
    Bm = w8[:, 1::2, :].reshape(128, 4, 8, 128)
    inter = np.stack([A[..., ::-1], Bm[..., ::-1]], axis=-1)  # [p,cp,g,128,2]
    return np.ascontiguousarray(inter.reshape(128, 4, 8, 256))


def _build():
    nc = bacc.Bacc()

    xt_d = nc.dram_tensor("xt", [4, 128, 8, 512], BF16, kind="ExternalInput")
    xt8_d = nc.dram_tensor("xt8", [4, 128, 8, 512], FP8, kind="ExternalInput")
    wq8_d = nc.dram_tensor("wq8i", [128, 4, G, 256], FP8, kind="ExternalInput")
    wk8_d = nc.dram_tensor("wk8i", [128, 4, G, 256], FP8, kind="ExternalInput")
    wv_d = nc.dram_tensor("wv", [128, 8, D], BF16, kind="ExternalInput")
    bqs_d = nc.dram_tensor("bqs", [128, G], F32, kind="ExternalInput")
    bks_d = nc.dram_tensor("bks", [128, G], F32, kind="ExternalInput")
    bvt_d = nc.dram_tensor("bvt", [1, D], BF16, kind="ExternalInput")
    out_d = nc.dram_tensor("ctxT", [NP, 2, 128, 4, CN], BF16,
                           kind="ExternalOutput")

    with tile.TileContext(nc) as tc:
        with (
            tc.tile_pool(name="big", bufs=1) as big,
            tc.tile_pool(name="small", bufs=1) as small,
            tc.tile_pool(name="ework", bufs=3) as ework,
            tc.tile_pool(name="zwork", bufs=2) as zwork,
            tc.tile_pool(name="sc", bufs=2, space="PSUM") as scp,
            tc.tile_pool(name="cx", bufs=1, space="PSUM") as cxp,
        ):
            proj8 = tc.alloc_tile_pool(name="proj8", bufs=1)
            # ---- load inputs: chunk-major contiguous layouts, 3 queues ------
            xt_s = big.tile([128, 4, 8, 512], BF16)  # [p, mc, dt, mcol]
            xt8_s = proj8.tile([128, 4, 8, 512], FP8)
            wq8_s = proj8.tile([128, 4, G, 256], FP8)
            wk8_s = big.tile([128, 4, G, 256], FP8)
            wv_s = big.tile([128, 8, D], BF16)
            # sync queue: Q-critical then the rest of x8 (merged transfers
            # so the per-queue DMA semaphore pool is never reused, which
            # would serialize later loads behind false dependencies)
            nc.sync.dma_start(xt8_s[:, 0], xt8_d[0])
            nc.sync.dma_start(wq8_s[:], wq8_d[:])
            nc.sync.dma_start(xt8_s[:, 1:4], xt8_d[1:4].rearrange("c p t m -> p c t m"))
            nc.sync.dma_start(xt_s[:, 2:4], xt_d[2:4].rearrange("c p t m -> p c t m"))
            # scalar queue: K weights, then x bf16 for early V tiles
            nc.scalar.dma_start(wk8_s[:], wk8_d[:])
            nc.scalar.dma_start(xt_s[:, 0:2], xt_d[0:2].rearrange("c p t m -> p c t m"))
            # gpsimd queue: biases + V weights
            bqs_s = small.tile([128, G], F32)
            nc.gpsimd.dma_start(bqs_s[:], bqs_d[:])
            bks_s = small.tile([128, G], F32)
            nc.gpsimd.dma_start(bks_s[:], bks_d[:])
            bvt_s = small.tile([1, D], BF16)
            nc.gpsimd.dma_start(bvt_s[:], bvt_d[:])
            wv_s_dma = nc.gpsimd.dma_start(wv_s[:], wv_d[:])
            ones_s = small.tile([1, 128], BF16)
            nc.vector.memset(ones_s[:], 1.0)

            kt_s = big.tile([128, G, S], BF16)       # [h, g, m]
            v_s = big.tile([128, MT, D], BF16)       # [m, mt, g*128+h]
            qt_s = big.tile([128, G, NQ], BF16)      # [h, g, n]
            ctxt_s = big.tile([128, G, CN], BF16)    # [h, g, n] one pass

            ident = mybir.ActivationFunctionType.Identity
            expf = mybir.ActivationFunctionType.Exp

            # ---- Q^T projection (queries are XT columns 0:NQ), fp8 DRSW -----
            for g in range(G):
                qp = scp.tile([128, NQ], F32, tag="sc")
                for cp in range(4):
                    nc.tensor.matmul(
                        qp[:],
                        wq8_s[:, cp, g, :],
                        xt8_s[:, 0, 2 * cp : 2 * cp + 2, :],
                        start=(cp == 0),
                        stop=(cp == 3),
                        perf_mode=DRSW,
                    )
                nc.scalar.activation(
                    qt_s[:, g, :], qp[:], ident,
                    bias=bqs_s[:, g : g + 1], scale=float(SCALE * DESC),
                )

            # ---- K/V production pieces --------------------------------------
            def k_part(mc, gh):
                # K^T columns mc*512 .. +512 for groups gh*4..+4: fp8
                # DoubleRow with software-interleaved weights (keeps FWL)
                for g in range(gh * 4, gh * 4 + 4):
                    kp = scp.tile([128, 512], F32, tag="sc")
                    for cp in range(4):
                        nc.tensor.matmul(
                            kp[:],
                            wk8_s[:, cp, g, :],
                            xt8_s[:, mc, 2 * cp : 2 * cp + 2, :],
                            start=(cp == 0),
                            stop=(cp == 3),
                            perf_mode=DRSW,
                        )
                    nc.scalar.activation(
                        kt_s[:, g, mc * 512 : (mc + 1) * 512], kp[:], ident,
                        bias=bks_s[:, g : g + 1], scale=float(DESC),
                    )

            def v_mt(mt):
                # V rows for one m-tile of 128 x full D
                for hc in range(2):
                    vp = scp.tile([128, 512], F32, tag="sc")
                    for dt in range(8):
                        nc.tensor.matmul(
                            vp[:],
                            xt_s[:, mt // 4, dt,
                                 (mt % 4) * 128 : (mt % 4) * 128 + 128],
                            wv_s[:, dt, hc * 512 : (hc + 1) * 512],
                            start=(dt == 0),
                            stop=False,
                        )
                    nc.tensor.matmul(
                        vp[:],
                        ones_s[:],
                        bvt_s[:, hc * 512 : (hc + 1) * 512],
                        start=False,
                        stop=True,
                    )
                    nc.scalar.activation(
                        v_s[:, mt, hc * 512 : (hc + 1) * 512], vp[:], ident
                    )

            def scores_softmax(np_, st, fill=None, pool=None, tag="e"):
                """Supertile: scores + exp + group-softmax for m-tiles
                2*st and 2*st+1 against n-chunk np_. E layout
                [128, g, sub*256+n]. fill(i) emits deferred PE work after
                each of the 4 exp stages. Returns the normalized E tile."""
                n0 = np_ * CN
                e_s = (pool or ework).tile([128, G, 2 * CN], BF16, tag=tag)
                t1 = zwork.tile([128, 4, 2 * CN], BF16, tag="t1", bufs=1)
                # stage order (sub, half): (0,0),(1,0),(0,1),(1,1) so the
                # half-0 pair-sum (gpsimd) can start at the halfway point
                for i in range(4):
                    half, sub = divmod(i, 2)
                    mt = 2 * st + sub
                    sp = scp.tile([128, 4, CN], F32, tag="sc")
                    for gl in range(4):
                        g = half * 4 + gl
                        nc.tensor.matmul(
                            sp[:, gl, :],
                            kt_s[:, g, mt * 128 : (mt + 1) * 128],
                            qt_s[:, g, n0 : n0 + CN],
                            start=True,
                            stop=True,
                        )
                    nc.scalar.activation(
                        e_s[:, half * 4 : half * 4 + 4, sub * CN : (sub + 1) * CN],
                        sp[:], expf,
                    )
                    # pair-sums: half 0 on gpsimd (slack before t2 needs it),
                    # half 1 on vector (fast, feeds t2 immediately)
                    if i == 1:
                        nc.gpsimd.tensor_add(
                            t1[:, 0:2, :], e_s[:, 0:2, :], e_s[:, 2:4, :]
                        )
                    elif i == 3:
                        nc.vector.tensor_add(
                            t1[:, 2:4, :], e_s[:, 4:6, :], e_s[:, 6:8, :]
                        )
                    if fill is not None:
                        fill(i)
                t2 = zwork.tile([128, 2, 2 * CN], BF16, tag="t2", bufs=1)
                nc.vector.tensor_add(t2[:], t1[:, 0:2, :], t1[:, 2:4, :])
                z32 = zwork.tile([128, 2 * CN], F32, tag="z32", bufs=1)
                nc.vector.tensor_add(z32[:], t2[:, 0, :], t2[:, 1, :])
                nc.vector.reciprocal_approx_fast(out=z32[:], in_=z32[:])
                wb = zwork.tile([128, 2 * CN], BF16, tag="wb", bufs=1)
                nc.vector.tensor_copy(wb[:], z32[:])
                # normalize per sub-tile so ctx matmuls on sub 0 can start
                # while sub 1 is still being scaled
                for sub in range(2):
                    wb_b = bass.AP(
                        tensor=wb.tensor, offset=wb.offset + sub * CN,
                        ap=[wb.ap[0], [0, G], [1, CN]],
                    )
                    nc.vector.tensor_mul(
                        e_s[:, :, sub * CN : (sub + 1) * CN],
                        e_s[:, :, sub * CN : (sub + 1) * CN],
                        wb_b,
                    )
                return e_s

            def ctx_mms(st, e_s, ctx_acc, sub, gs=range(G), stop=False):
                # ctx^T accumulation: out[h, n] += V_g^T @ P_g^T
                # Two groups share each 2KB PSUM bank. start=True resets the
                # whole bank's has_written bits, so only the first group of
                # each bank pair may issue it; the second group's first write
                # lands on cleared bits and overwrites, later writes accumulate.
                mt = 2 * st + sub
                for g in gs:
                    nc.tensor.matmul(
                        ctx_acc[:, g, :],
                        v_s[:, mt, g * 128 : (g + 1) * 128],
                        e_s[:, g, sub * CN : (sub + 1) * CN],
                        start=(mt == 0 and g % 2 == 0),
                        stop=stop,
                        skip_group_check=True,
                    )

            def drain_and_evac(np_, ctx_acc, pend, pre):
                # drain order: all ready supertiles first, the freshest
                # (whose softmax chain may still be running) last, so the
                # PE always has ready ctx work while the last chain ends.
                order = pend[:-1] + sorted(pre.items()) + pend[-1:]
                for gh in range(2):
                    gs = range(gh * 4, gh * 4 + 4)
                    for st_, e_ in order:
                        for sub in range(2):
                            nc_stop = (st_, sub) == (order[-1][0], 1)
                            ctx_mms(st_, e_, ctx_acc, sub, gs=gs, stop=nc_stop)
                    nc.scalar.activation(
                        ctxt_s[:, gh * 4 : gh * 4 + 4, :],
                        ctx_acc[:, gh * 4 : gh * 4 + 4, :], ident,
                    )
                    nc.sync.dma_start(
                        out_d[np_, gh], ctxt_s[:, gh * 4 : gh * 4 + 4, :]
                    )

            def emit_pass(np_, ctx_acc, with_kv, n_fresh, pre, pre_out):
                # software pipeline over supertiles: ctx matmuls for
                # supertile st are emitted across the 4 exp stages of
                # supertile st+depth, hiding the exp->tree->mul chain.
                # In pass 0 the K/V production is spread evenly: each
                # supertile produces its own 2 V m-tiles plus half the
                # groups of the NEXT 512-col K chunk (consumed 2 supers
                # later). Pass 0 supertiles >= ST-NPRE also precompute
                # pass 1's softmax into retained E tiles.
                depth = 1 if with_kv else 2
                pend = []
                # K parts front-loaded: chunk 1 at st 0, then one part per
                # supertile; all K done by st 4 so the fp8 operand pool can
                # be recycled before the first pass-1 precompute.
                kparts = [(1, 0), (1, 1), (2, 0), (2, 1), (3, 0), (3, 1)]
                ksched = {0: kparts[0:2], 1: kparts[2:3], 2: kparts[3:4],
                          3: kparts[4:5], 4: kparts[5:6]}
                for st in range(n_fresh):
                    if with_kv:
                        v_mt(2 * st)
                        for mc, gh in ksched.get(st, []):
                            k_part(mc, gh)
                        v_mt(2 * st + 1)
                        if st == 4:
                            # fp8 projection operands are dead once the
                            # last K part is emitted; recycle their SBUF
                            # for the retained pass-1 E tiles.
                            proj8.release()
                            pre_out["pool"] = tc.alloc_tile_pool(
                                name="epre", bufs=NPRE
                            )

                    def fill(i, _p=(pend[0] if len(pend) == depth else None)):
                        if _p is not None:
                            half, sub = divmod(i, 2)
                            ctx_mms(_p[0], _p[1], ctx_acc, sub,
                                    gs=range(half * 4, half * 4 + 4))

                    e_s = scores_softmax(np_, st, fill)
                    if len(pend) == depth:
                        pend.pop(0)
                    pend.append((st, e_s))
                    if pre_out is not None and st >= ST - NPRE:
                        pre_out[st] = scores_softmax(
                            1, st, pool=pre_out["pool"], tag="epre"
                        )
                return pend

            # prologue: K chunk 0 so pass-0 supertile 0 can score
            k_part(0, 0)
            k_part(0, 1)
            epre = {}
            ctx_acc = cxp.tile([128, G, CN], F32, tag="cx")
            pend = emit_pass(0, ctx_acc, True, ST, {}, epre)
            epool = epre.pop("pool")
            drain_and_evac(0, ctx_acc, pend, {})
            ctx_acc = cxp.tile([128, G, CN], F32, tag="cx")
            pend = emit_pass(1, ctx_acc, False, ST - NPRE, epre, None)
            drain_and_evac(1, ctx_acc, pend, epre)
            epool.release()

    nc.compile()
    return nc


def _prep_inputs(hidden_states, Wq, bq, Wk, bk, Wv, bv):
    bf = ml_dtypes.bfloat16
    f8 = ml_dtypes.float8_e4m3
    # wv rearranged to the on-chip [p, t, o] layout (d = t*128 + p)
    wv_b = np.ascontiguousarray(
        np.asarray(Wv, np.float32).reshape(8, 128, D).transpose(1, 0, 2)
    ).astype(bf)

    # Wq/Wk scaled fp8, rearranged [d, o] -> [p, t, o], then
    # software-interleaved for DoubleRowSwInterleave
    def prep_w8(W):
        w8 = (np.asarray(W, np.float32) * WS).astype(f8)
        return _sw_interleave(w8.reshape(8, 128, D).transpose(1, 0, 2))

    wq8i = prep_w8(Wq)
    wk8i = prep_w8(Wk)
    bqs = np.ascontiguousarray(
        (np.asarray(bq, np.float32) * SCALE).reshape(G, 128).T
    )
    bks = np.ascontiguousarray(np.asarray(bk, np.float32).reshape(G, 128).T)
    bvt = np.asarray(bv, np.float32).astype(bf).reshape(1, D)

    in_maps = []
    for core in range(8):
        b, j = divmod(core, 4)
        xt = np.asarray(hidden_states[b], np.float32).T  # (D, S)
        xt = np.roll(xt, -j * NQ, axis=1)                # queries first
        # chunk-major on-chip layout [mc, p, t, mcol] (d = t*128 + p)
        xtp = np.ascontiguousarray(
            xt.reshape(8, 128, 4, 512).transpose(2, 1, 0, 3)
        )
        in_maps.append(
            {
                "xt": xtp.astype(bf),
                "xt8": (xtp * XS).astype(f8),
                "wq8i": wq8i, "wk8i": wk8i, "wv": wv_b,
                "bqs": bqs, "bks": bks, "bvt": bvt,
            }
        )
    return in_maps


def kernel(hidden_states, Wq, bq, Wk, bk, Wv, bv, _trace=False, _tmpdir=None):
    if "nc" not in _CACHE:
        _CACHE["nc"] = _build()
    nc = _CACHE["nc"]
    in_maps = _prep_inputs(hidden_states, Wq, bq, Wk, bk, Wv, bv)
    res = run_bass_kernel_spmd(
        nc, in_maps, list(range(8)), trace=_trace,
        **({"tmpdir": _tmpdir} if _tmpdir else {}),
    )
    _CACHE["last_result"] = res
    out = np.empty((B, S, D), np.float32)
    for core in range(8):
        b, j = divmod(core, 4)
        # ctxT [np, gh, p, gl, n]: d = (gh*4+gl)*128 + p, row = np*CN + n
        ct = res.results[core]["ctxT"].astype(np.float32)
        blk = ct.transpose(0, 4, 1, 3, 2).reshape(NQ, D)
        out[b, j * NQ : (j + 1) * NQ, :] = blk
    return out
